# revision 9
# baseline (speedup 1.0000x reference)
"""Per-patch dynamic conv (nn_DynaMicConv) as a Bass/Tile kernel on 8 TRN2 cores.

Math: for each patch p of a 14x14 grid over a 224x224 image, out[b, :, p] =
W[p] @ patch_pixels[b, p] + bias[p], i.e. 196 independent [64,768] x [768,768]
matmuls. DMA-bound: the f16 weight stack is 231 MB and every byte is read once.

Sharding: patch-parallel, exactly balanced. Each core gets 24 full patches +
one half patch (COUT split 384/384 between a core pair): 8 x 24.5 = 196, no
padding, every core moves the same 33.8 MB.

DMA granularity is per patch: one dma_start per patch delivers that patch's W
(lhsT-chunks) AND its pixels in a single [128, 9984B] transfer (one 64KB-max
descriptor per partition row, striped uniformly over the 16 SDMA engines).
Fine granularity keeps the stream continuous: the PE starts after the first
patch lands (~3.5us) instead of after a 6.4MB group (~18us), and buffer-free
backpressure is per patch, so the sync-queue never drains mid-run. Bias for
all patches loads once up-front as a [16, 150*16] tile (16 descriptors -- the
old per-group [1, N] bias DMAs were single-descriptor transfers that all
landed on one engine, the same engine that carries the stream's receipts,
making it a ~10us straggler; the [1, 25*768] row is auto-split into 16
descriptors). The ones vector is memset on-chip, not DMA'd.

Compute per patch: PSUM[64, 512|256] = sum_kc lhsT_kc.T @ rhs_kc with bias
pre-loaded via a ones[1,64] stationary matmul (start=True). PSUM -> SBUF copy
(cast to f16) on DVE; outputs stage in per-segment SBUF tiles and store in
STORE_CUTS chunks on the scalar ring.
"""

import numpy as np

import concourse.bacc as bacc
import concourse.mybir as mybir
import concourse.tile as tile
from concourse.bass_utils import run_bass_kernel_spmd

B, CIN, IMG, PS, G = 64, 3, 224, 16, 14
P = G * G                 # 196 patches
COUT = 768
K = CIN * PS * PS         # 768 contraction
KCH = K // 128            # 6 k-chunks
NCORES = 8
NFULL = 24                # full patches per core
HCOUT = COUT // 2         # half-patch output channels (384)
NP_C = NFULL + 1          # per-core patch slots (last one is the half patch)
WCOLS = KCH * COUT + KCH * B        # 4992 f16 cols per full-patch row
HCOLS = KCH * HCOUT + KCH * B       # 2688 f16 cols per half-patch row
OCOLS = NFULL * COUT + HCOUT        # 18816 output cols per core

F32 = mybir.dt.float32

MODE = "f16"
_DTYPES = {
    "f32r": (mybir.dt.float32r, np.float32),
    "f16": (mybir.dt.float16, np.float16),
    "bf16": (mybir.dt.bfloat16, None),
}
OUT_F16 = True

WBUFS = 8    # per-patch W+x tiles in flight
# output store split points (patch indices); the final two segments are small
# (one full patch, then just the half patch) so the last stores are tiny and
# the post-stream tail stays short
STORE_CUTS = [0, 5, 10, 15, 20, 23, NFULL, NP_C]

TRACE = False
TRACE_CORES = [0]
LAST_RESULT = None

_CACHE = {}


def _np_dtype(mode):
    mdt, ndt = _DTYPES[mode]
    if ndt is None:
        import ml_dtypes
        ndt = ml_dtypes.bfloat16
    return mdt, ndt


def _seg_cols(seg):
    """Output column extent of store segment `seg`."""
    lo, hi = STORE_CUTS[seg], STORE_CUTS[seg + 1]
    ncols = 0
    for p in range(lo, hi):
        ncols += COUT if p < NFULL else HCOUT
    return lo * COUT, ncols


def _build(mode):
    mdt, _ = _np_dtype(mode)
    odt = mybir.dt.float16 if OUT_F16 else F32
    nc = bacc.Bacc("TRN2", target_bir_lowering=False, debug=False)
    wf_d = nc.dram_tensor("wf", [NFULL, 128, WCOLS], mdt, kind="ExternalInput")
    wh_d = nc.dram_tensor("wh", [128, HCOLS], mdt, kind="ExternalInput")
    b_d = nc.dram_tensor("bs", [1, NP_C * COUT], mdt, kind="ExternalInput")
    o_d = nc.dram_tensor("out", [B, OCOLS], odt, kind="ExternalOutput")

    with tile.TileContext(nc) as tc:
        with (
            tc.tile_pool(name="const", bufs=1) as cpool,
            tc.tile_pool(name="wp", bufs=WBUFS) as wpool,
            tc.tile_pool(name="op", bufs=3) as opool,
            tc.tile_pool(name="ps", bufs=3, space="PSUM") as pspool,
        ):
            ones = cpool.tile([1, B], mdt)
            nc.gpsimd.memset(ones[:], 1.0)
            bt = cpool.tile([1, NP_C * COUT], mdt)
            nc.scalar.dma_start(bt[:], b_d[:])

            seg = 0
            oseg = None
            ocol = 0
            for p in range(NP_C):
                full = p < NFULL
                wt = wpool.tile([128, WCOLS], mdt, tag="w")
                if full:
                    nc.sync.dma_start(wt[:], wf_d[p])
                else:
                    nc.sync.dma_start(wt[:, :HCOLS], wh_d[:])

                cw = COUT if full else HCOUT
                xbase = KCH * cw
                bcol = p * COUT
                ps = pspool.tile([B, COUT], F32, tag="ps", bufs=4)
                h1 = 512 if full else HCOUT
                nc.tensor.matmul(ps[:, :h1], ones[:],
                                 bt[:, bcol: bcol + h1],
                                 start=True, stop=False)
                if full:
                    nc.tensor.matmul(ps[:, 512: COUT], ones[:],
                                     bt[:, bcol + 512: bcol + COUT],
                                     start=True, stop=False)
                for kc in range(KCH):
                    lhs = wt[:, xbase + kc * B: xbase + (kc + 1) * B]
                    last = kc == KCH - 1
                    nc.tensor.matmul(ps[:, :h1], lhs,
                                     wt[:, kc * cw: kc * cw + h1],
                                     start=False, stop=last)
                    if full:
                        nc.tensor.matmul(ps[:, 512: COUT], lhs,
                                         wt[:, kc * cw + 512: (kc + 1) * cw],
                                         start=False, stop=last)

                if p == STORE_CUTS[seg]:
                    _, ncols = _seg_cols(seg)
                    oseg = opool.tile([B, ncols], odt, tag="o", name=f"oseg{seg}")
                    ocol = 0
                nc.vector.tensor_copy(oseg[:, ocol: ocol + cw], ps[:, :cw])
                ocol += cw
                if p + 1 == STORE_CUTS[seg + 1]:
                    base, ncols = _seg_cols(seg)
                    nc.scalar.dma_start(o_d[:, base: base + ncols], oseg[:])
                    seg += 1
    nc.compile()
    return nc


def _prep(x, W, b, mode):
    _, ndt = _np_dtype(mode)
    # patch pixels, k-transposed: xp[p, k, b] with k = c*256 + r*16 + s
    xp = (x.reshape(B, CIN, G, PS, G, PS)
           .transpose(2, 4, 1, 3, 5, 0)
           .reshape(P, K, B))
    # -> [P, 128(kpart), KCH*B] (kc-major within each partition row)
    xr = (xp.reshape(P, KCH, 128, B).transpose(0, 2, 1, 3)
            .reshape(P, 128, KCH * B).astype(ndt))
    # weights: wr[p, kpart, kc*COUT + o] = W[p, o, kc*128 + kpart]
    Wm = W.reshape(P, COUT, KCH, 128)
    wr = (Wm.transpose(0, 3, 2, 1).reshape(P, 128, KCH * COUT).astype(ndt))
    br = b.astype(ndt)

    in_maps = []
    for c in range(NCORES):
        base = c * NFULL
        sp = 192 + c // 2                       # shared patch index
        olo = 0 if c % 2 == 0 else HCOUT        # cout slice of the half
        wf = np.concatenate([wr[base: base + NFULL],
                             xr[base: base + NFULL]], axis=2)
        # half patch: W cols [r, kc*HCOUT + o] for o in the slice
        wh_w = (Wm[sp, olo: olo + HCOUT]        # [384, KCH, 128]
                .transpose(2, 1, 0).reshape(128, KCH * HCOUT).astype(ndt))
        wh = np.concatenate([wh_w, xr[sp]], axis=1)
        bs = np.zeros((1, NP_C * COUT), dtype=ndt)
        for p in range(NP_C):
            gp = base + p if p < NFULL else sp
            cw = COUT if p < NFULL else HCOUT
            off = olo if p == NFULL else 0
            bs[0, p * COUT: p * COUT + cw] = br[gp, off: off + cw]
        in_maps.append({
            "wf": np.ascontiguousarray(wf),
            "wh": np.ascontiguousarray(wh),
            "bs": bs,
        })
    return in_maps


def kernel(x, W, b):
    global LAST_RESULT
    x = np.ascontiguousarray(np.asarray(x, dtype=np.float32))
    W = np.ascontiguousarray(np.asarray(W, dtype=np.float32))
    b = np.ascontiguousarray(np.asarray(b, dtype=np.float32))
    in_maps = _prep(x, W, b, MODE)
    key = ("nc", MODE, OUT_F16, WBUFS, tuple(STORE_CUTS))
    if key not in _CACHE:
        _CACHE[key] = _build(MODE)
    res = run_bass_kernel_spmd(
        _CACHE[key], in_maps, core_ids=list(range(NCORES)),
        trace=TRACE, trace_cores=TRACE_CORES,
    )
    LAST_RESULT = res
    # assemble [B, P, COUT]
    out = np.empty((B, P, COUT), dtype=np.float32)
    for c in range(NCORES):
        oc = res.results[c]["out"].astype(np.float32)   # [B, OCOLS]
        base = c * NFULL
        out[:, base: base + NFULL] = oc[:, : NFULL * COUT].reshape(B, NFULL, COUT)
        sp = 192 + c // 2
        olo = 0 if c % 2 == 0 else HCOUT
        out[:, sp, olo: olo + HCOUT] = oc[:, NFULL * COUT:]
    return np.ascontiguousarray(out.transpose(0, 2, 1)).reshape(B, COUT, G, G)


# revision 11
# speedup vs baseline: 1.0018x; 1.0018x over previous
"""Per-patch dynamic conv (nn_DynaMicConv) as a Bass/Tile kernel on 8 TRN2 cores.

Math: for each patch p of a 14x14 grid over a 224x224 image, out[b, :, p] =
W[p] @ patch_pixels[b, p] + bias[p], i.e. 196 independent [64,768] x [768,768]
matmuls. DMA-bound: the f16 weight stack is 231 MB and every byte is read once.

Sharding: patch-parallel, exactly balanced. Each core gets 24 full patches +
one half patch (COUT split 384/384 between a core pair): 8 x 24.5 = 196, no
padding, every core moves the same 33.8 MB.

DMA granularity is per patch: one dma_start per patch delivers that patch's W
(lhsT-chunks) AND its pixels in a single [128, 9984B] transfer (one 64KB-max
descriptor per partition row, striped uniformly over the 16 SDMA engines).
Fine granularity keeps the stream continuous: the PE starts after the first
patch lands (~3.5us) instead of after a 6.4MB group (~18us), and buffer-free
backpressure is per patch, so the sync-queue never drains mid-run. Bias for
all patches loads once up-front as a [16, 150*16] tile (16 descriptors -- the
old per-group [1, N] bias DMAs were single-descriptor transfers that all
landed on one engine, the same engine that carries the stream's receipts,
making it a ~10us straggler; the [1, 25*768] row is auto-split into 16
descriptors). The ones vector is memset on-chip, not DMA'd.

Compute per patch: PSUM[64, 512|256] = sum_kc lhsT_kc.T @ rhs_kc with bias
pre-loaded via a ones[1,64] stationary matmul (start=True). PSUM -> SBUF copy
(cast to f16) on DVE; outputs stage in per-segment SBUF tiles and store in
STORE_CUTS chunks on the scalar ring.
"""

import numpy as np

import concourse.bacc as bacc
import concourse.mybir as mybir
import concourse.tile as tile
from concourse.bass_utils import run_bass_kernel_spmd

B, CIN, IMG, PS, G = 64, 3, 224, 16, 14
P = G * G                 # 196 patches
COUT = 768
K = CIN * PS * PS         # 768 contraction
KCH = K // 128            # 6 k-chunks
NCORES = 8
NFULL = 24                # full patches per core
HCOUT = COUT // 2         # half-patch output channels (384)
NP_C = NFULL + 1          # per-core patch slots (last one is the half patch)
WCOLS = KCH * COUT + KCH * B        # 4992 f16 cols per full-patch row
HCOLS = KCH * HCOUT + KCH * B       # 2688 f16 cols per half-patch row
OCOLS = NFULL * COUT + HCOUT        # 18816 output cols per core

F32 = mybir.dt.float32

MODE = "f16"
_DTYPES = {
    "f32r": (mybir.dt.float32r, np.float32),
    "f16": (mybir.dt.float16, np.float16),
    "bf16": (mybir.dt.bfloat16, None),
}
OUT_F16 = True

WBUFS = 10   # per-patch W+x tiles in flight
# output store split points (patch indices); the final two segments are small
# (one full patch, then just the half patch) so the last stores are tiny and
# the post-stream tail stays short
STORE_CUTS = [0, 5, 10, 15, 20, 23, NFULL, NP_C]

TRACE = False
TRACE_CORES = [0]
LAST_RESULT = None

_CACHE = {}


def _np_dtype(mode):
    mdt, ndt = _DTYPES[mode]
    if ndt is None:
        import ml_dtypes
        ndt = ml_dtypes.bfloat16
    return mdt, ndt


def _seg_cols(seg):
    """Output column extent of store segment `seg`."""
    lo, hi = STORE_CUTS[seg], STORE_CUTS[seg + 1]
    ncols = 0
    for p in range(lo, hi):
        ncols += COUT if p < NFULL else HCOUT
    return lo * COUT, ncols


def _build(mode):
    mdt, _ = _np_dtype(mode)
    odt = mybir.dt.float16 if OUT_F16 else F32
    nc = bacc.Bacc("TRN2", target_bir_lowering=False, debug=False)
    wf_d = nc.dram_tensor("wf", [NFULL, 128, WCOLS], mdt, kind="ExternalInput")
    wh_d = nc.dram_tensor("wh", [128, HCOLS], mdt, kind="ExternalInput")
    b_d = nc.dram_tensor("bs", [1, NP_C * COUT], mdt, kind="ExternalInput")
    o_d = nc.dram_tensor("out", [B, OCOLS], odt, kind="ExternalOutput")

    with tile.TileContext(nc) as tc:
        with (
            tc.tile_pool(name="const", bufs=1) as cpool,
            tc.tile_pool(name="wp", bufs=WBUFS) as wpool,
            tc.tile_pool(name="op", bufs=3) as opool,
            tc.tile_pool(name="ps", bufs=3, space="PSUM") as pspool,
        ):
            ones = cpool.tile([1, B], mdt)
            nc.gpsimd.memset(ones[:], 1.0)
            bt = cpool.tile([1, NP_C * COUT], mdt)
            nc.scalar.dma_start(bt[:], b_d[:])

            seg = 0
            oseg = None
            ocol = 0
            for p in range(NP_C):
                full = p < NFULL
                wt = wpool.tile([128, WCOLS], mdt, tag="w")
                if full:
                    nc.sync.dma_start(wt[:], wf_d[p])
                else:
                    nc.sync.dma_start(wt[:, :HCOLS], wh_d[:])

                cw = COUT if full else HCOUT
                xbase = KCH * cw
                bcol = p * COUT
                ps1 = pspool.tile([B, 512], F32, tag="ps1", bufs=4)
                if full:
                    ps2 = pspool.tile([B, 256], F32, tag="ps2")
                    nc.tensor.matmul(ps1[:], ones[:],
                                     bt[:, bcol: bcol + 512],
                                     start=True, stop=False)
                    nc.tensor.matmul(ps2[:], ones[:],
                                     bt[:, bcol + 512: bcol + COUT],
                                     start=True, stop=False)
                else:
                    nc.tensor.matmul(ps1[:, :HCOUT], ones[:],
                                     bt[:, bcol: bcol + HCOUT],
                                     start=True, stop=False)
                for kc in range(KCH):
                    lhs = wt[:, xbase + kc * B: xbase + (kc + 1) * B]
                    last = kc == KCH - 1
                    if full:
                        nc.tensor.matmul(ps1[:], lhs,
                                         wt[:, kc * cw: kc * cw + 512],
                                         start=False, stop=last)
                        nc.tensor.matmul(ps2[:], lhs,
                                         wt[:, kc * cw + 512: (kc + 1) * cw],
                                         start=False, stop=last)
                    else:
                        nc.tensor.matmul(ps1[:, :HCOUT], lhs,
                                         wt[:, kc * cw: (kc + 1) * cw],
                                         start=False, stop=last)

                if p == STORE_CUTS[seg]:
                    _, ncols = _seg_cols(seg)
                    oseg = opool.tile([B, ncols], odt, tag="o", name=f"oseg{seg}")
                    ocol = 0
                if full:
                    nc.vector.tensor_copy(oseg[:, ocol: ocol + 512], ps1[:])
                    nc.vector.tensor_copy(oseg[:, ocol + 512: ocol + COUT], ps2[:])
                    ocol += COUT
                else:
                    nc.vector.tensor_copy(oseg[:, ocol: ocol + HCOUT],
                                          ps1[:, :HCOUT])
                    ocol += HCOUT
                if p + 1 == STORE_CUTS[seg + 1]:
                    base, ncols = _seg_cols(seg)
                    nc.scalar.dma_start(o_d[:, base: base + ncols], oseg[:])
                    seg += 1
    nc.compile()
    return nc


def _prep(x, W, b, mode):
    _, ndt = _np_dtype(mode)
    # patch pixels, k-transposed: xp[p, k, b] with k = c*256 + r*16 + s
    xp = (x.reshape(B, CIN, G, PS, G, PS)
           .transpose(2, 4, 1, 3, 5, 0)
           .reshape(P, K, B))
    # -> [P, 128(kpart), KCH*B] (kc-major within each partition row)
    xr = (xp.reshape(P, KCH, 128, B).transpose(0, 2, 1, 3)
            .reshape(P, 128, KCH * B).astype(ndt))
    # weights: wr[p, kpart, kc*COUT + o] = W[p, o, kc*128 + kpart]
    Wm = W.reshape(P, COUT, KCH, 128)
    wr = (Wm.transpose(0, 3, 2, 1).reshape(P, 128, KCH * COUT).astype(ndt))
    br = b.astype(ndt)

    in_maps = []
    for c in range(NCORES):
        base = c * NFULL
        sp = 192 + c // 2                       # shared patch index
        olo = 0 if c % 2 == 0 else HCOUT        # cout slice of the half
        wf = np.concatenate([wr[base: base + NFULL],
                             xr[base: base + NFULL]], axis=2)
        # half patch: W cols [r, kc*HCOUT + o] for o in the slice
        wh_w = (Wm[sp, olo: olo + HCOUT]        # [384, KCH, 128]
                .transpose(2, 1, 0).reshape(128, KCH * HCOUT).astype(ndt))
        wh = np.concatenate([wh_w, xr[sp]], axis=1)
        bs = np.zeros((1, NP_C * COUT), dtype=ndt)
        for p in range(NP_C):
            gp = base + p if p < NFULL else sp
            cw = COUT if p < NFULL else HCOUT
            off = olo if p == NFULL else 0
            bs[0, p * COUT: p * COUT + cw] = br[gp, off: off + cw]
        in_maps.append({
            "wf": np.ascontiguousarray(wf),
            "wh": np.ascontiguousarray(wh),
            "bs": bs,
        })
    return in_maps


def kernel(x, W, b):
    global LAST_RESULT
    x = np.ascontiguousarray(np.asarray(x, dtype=np.float32))
    W = np.ascontiguousarray(np.asarray(W, dtype=np.float32))
    b = np.ascontiguousarray(np.asarray(b, dtype=np.float32))
    in_maps = _prep(x, W, b, MODE)
    key = ("nc", MODE, OUT_F16, WBUFS, tuple(STORE_CUTS))
    if key not in _CACHE:
        _CACHE[key] = _build(MODE)
    res = run_bass_kernel_spmd(
        _CACHE[key], in_maps, core_ids=list(range(NCORES)),
        trace=TRACE, trace_cores=TRACE_CORES,
    )
    LAST_RESULT = res
    # assemble [B, P, COUT]
    out = np.empty((B, P, COUT), dtype=np.float32)
    for c in range(NCORES):
        oc = res.results[c]["out"].astype(np.float32)   # [B, OCOLS]
        base = c * NFULL
        out[:, base: base + NFULL] = oc[:, : NFULL * COUT].reshape(B, NFULL, COUT)
        sp = 192 + c // 2
        olo = 0 if c % 2 == 0 else HCOUT
        out[:, sp, olo: olo + HCOUT] = oc[:, NFULL * COUT:]
    return np.ascontiguousarray(out.transpose(0, 2, 1)).reshape(B, COUT, G, G)
